# revision 1
# baseline (speedup 1.0000x reference)
"""Trainium2 Bass kernel for ChannelAttention-SNN (LIF -> GAP -> 1x1conv -> BN
-> 1x1conv -> BN).

Contract: kernel(**inputs) takes the FULL unsharded inputs (as produced by
setup_inputs) and returns the FULL output [T, B, C, 1] float32.

Strategy (hardcoded for T=4, B=16, C=512, N=1024, Cr=64, 8 cores):
  - Data-parallel over B: core m processes b in {2m, 2m+1}.
  - LIF scan in "W-space": W_t = 2 v_t, so spike <=> W >= 2 at every step and
      W_t = W_{t-1} * md_{t-1} + x_t,   md = (W < 2) * 0.5,
    i.e. the 1/tau decay is folded into the 0/0.5 mask that the DVE
    tensor_scalar produces anyway while counting non-spikes (accum_out).
    The "+ x_t" runs inside the x-load itself: a gpsimd (SWDGE) DMA with
    accum_op=add and fp32->bf16 cast accumulates the raw x tile straight
    into the W state, so streaming costs the DVE only ~1.6us/tile
    (mask+count at 4x, mask-multiply at 2x) against the 2.93us DMA pace.
  - h1 = g @ w1.T partial rows are accumulated per timestep during the
    stream (PE is otherwise idle); one AllGather shares the 8 local rows.
  - Tail in channel-partition layout: h2.T blocks [128c x 64rows] so both
    BN2 statistics (ACT Square+accum) and the final normalization
    (per-partition tensor_scalar) need no transposes or broadcasts. The
    output leaves the device as [4, 128, 64] (c-major) and is transposed
    on the host.
  - repeat>1 emits the full pass back-to-back (interleaved, double-buffered
    pools) for work-scaling measurement; the real kernel uses repeat=1.
"""

import numpy as np

import concourse.bacc as bacc
import concourse.bass as bass
import concourse.mybir as mybir
import concourse.tile as tile
from concourse.bass_utils import run_bass_kernel_spmd
from concourse.masks import make_identity

T, B, C, N, CR = 4, 16, 512, 1024, 64
NCORES = 8
BL = B // NCORES            # batch rows per core (2)
CB = C // 128               # 128-partition channel blocks (4)
CG = C // 128               # output channel groups (4)
ROWS = T * BL               # local (t, b) rows (8)
TBALL = T * B               # total batch rows for BN (64)
BN_EPS = 1e-5

F32 = mybir.dt.float32
BF16 = mybir.dt.bfloat16
OP = mybir.AluOpType
AF = mybir.ActivationFunctionType


def _emit(tc, ctx, repeat=1, single=False, accum_dma=True, tail_stage=99):
    nc = tc.nc
    x = nc.dram_tensor("x", [T, BL, C, N], F32, kind="ExternalInput").ap()
    w1t = nc.dram_tensor("w1t", [C, CR], F32, kind="ExternalInput").ap()
    w2t = nc.dram_tensor("w2t", [CR, C], F32, kind="ExternalInput").ap()
    g1 = nc.dram_tensor("gamma1", [CR, 1], F32, kind="ExternalInput").ap()
    be1 = nc.dram_tensor("beta1", [CR, 1], F32, kind="ExternalInput").ap()
    g2g = nc.dram_tensor("gamma2g", [128, CG], F32, kind="ExternalInput").ap()
    be2g = nc.dram_tensor("beta2g", [128, CG], F32, kind="ExternalInput").ap()
    out = nc.dram_tensor("out", [CG, 128, TBALL], F32, kind="ExternalOutput").ap()

    consts = ctx.enter_context(tc.tile_pool(name="consts", bufs=1))
    wpool = ctx.enter_context(tc.tile_pool(name="wp", bufs=2))
    xpool = ctx.enter_context(tc.tile_pool(name="xp", bufs=2))
    mpool = ctx.enter_context(tc.tile_pool(name="mp", bufs=2))
    spool = ctx.enter_context(tc.tile_pool(name="sp", bufs=2))
    tpool = ctx.enter_context(tc.tile_pool(name="tp", bufs=2))
    opool = ctx.enter_context(tc.tile_pool(name="op", bufs=2))
    psum_h1 = ctx.enter_context(tc.tile_pool(name="ph1", bufs=2, space="PSUM"))
    psum_tr = ctx.enter_context(tc.tile_pool(name="ptr", bufs=2, space="PSUM"))
    psum_h2 = ctx.enter_context(tc.tile_pool(name="ph2", bufs=2, space="PSUM"))
    psum_c = ctx.enter_context(tc.tile_pool(name="pc", bufs=1, space="PSUM"))
    dram = ctx.enter_context(tc.tile_pool(name="dr", bufs=2, space="DRAM"))

    # ---- constants / weights ----
    ident = consts.tile([128, 128], F32, tag="id")
    make_identity(nc, ident)
    w1t_sb = consts.tile([128, CB, CR], F32, tag="w1")
    for cb in range(CB):
        nc.sync.dma_start(w1t_sb[:, cb, :], w1t[cb * 128:(cb + 1) * 128, :])
    w2t_sb = consts.tile([CR, C], F32, tag="w2")
    nc.sync.dma_start(w2t_sb[:], w2t[:])
    g1_sb = consts.tile([CR, 1], F32, tag="g1")
    nc.sync.dma_start(g1_sb[:], g1[:])
    be1_sb = consts.tile([CR, 1], F32, tag="be1")
    nc.sync.dma_start(be1_sb[:], be1[:])
    g2g_sb = consts.tile([128, CG], F32, tag="g2")
    nc.sync.dma_start(g2g_sb[:], g2g[:])
    be2g_sb = consts.tile([128, CG], F32, tag="be2")
    nc.sync.dma_start(be2g_sb[:], be2g[:])
    eps_sb = consts.tile([128, 1], F32, tag="eps")
    nc.vector.memset(eps_sb[:], BN_EPS)
    warm_sb = consts.tile([128, 1], F32, tag="warm")
    # warm the Sqrt/Square activation table before the timed body
    nc.scalar.activation(warm_sb[:], eps_sb[:], AF.Sqrt, bias=eps_sb[:])
    nc.scalar.activation(warm_sb[:], eps_sb[:], AF.Square)
    # BN1 guarantees mean(h1n) == beta1, so BN2's channel mean is known ahead
    # of time: mu2 = beta1 @ w2.T (+b2, which cancels), in [128c, CG] layout.
    mu2_ps = psum_c.tile([128, CG], F32, tag="mu2p", name="mu2_ps")
    for g in range(CG):
        nc.tensor.matmul(mu2_ps[:, g:g + 1],
                         w2t_sb[:, g * 128:(g + 1) * 128], be1_sb[:],
                         start=True, stop=True)
    mu2g = consts.tile([128, CG], F32, tag="mu2")
    nc.vector.tensor_copy(mu2g[:], mu2_ps[:])
    mu2sqg = consts.tile([128, CG], F32, tag="mu2sq")
    nc.vector.tensor_mul(mu2sqg[:], mu2g[:], mu2g[:])

    for _rep in range(repeat):
        # ---- streaming LIF + GAP counts + per-t h1 partial rows ----
        # W state ping-pongs between two tile sets across t (the accum DMA
        # adds x_t into the tile the mask-multiply just wrote).
        W = [[wpool.tile([128, BL, N], BF16, tag=f"W{s}{cb}", name=f"W{s}{cb}")
              for cb in range(CB)] for s in range(2)]
        stats = [spool.tile([128, CB, BL], F32, tag=f"st{t}", name=f"st{t}")
                 for t in range(T)]
        gm_all = spool.tile([128, CB, T, BL], F32, tag="gma", name="gm_all")
        for t in range(T):
            cur = W[t % 2]
            for cb in range(CB):
                src = x[t, :, cb * 128:(cb + 1) * 128, :].rearrange(
                    "b c n -> c b n")
                if accum_dma:
                    if t == 0:
                        nc.gpsimd.dma_start(cur[cb][:], src)
                    else:
                        nc.gpsimd.dma_start(cur[cb][:], src, accum_op=OP.add)
                else:
                    xt = xpool.tile([128, BL, N], F32, tag=f"x{cb}",
                                    name=f"xt{t}{cb}")
                    nc.sync.dma_start(xt[:], src)
                    if t == 0:
                        nc.scalar.activation(cur[cb][:], xt[:], AF.Copy,
                                             scale=1.0)
                    else:
                        y = xpool.tile([128, BL, N], BF16, tag=f"y{cb}",
                                       name=f"y{t}{cb}")
                        nc.scalar.activation(y[:], xt[:], AF.Copy, scale=1.0)
                        nc.vector.tensor_add(cur[cb][:], cur[cb][:], y[:])
                m = mpool.tile([128, BL, N], BF16, tag=f"m{cb}", name=f"m{t}{cb}")
                for b in range(BL):
                    nc.vector.tensor_scalar(
                        out=m[:, b, :],
                        in0=cur[cb][:, b, :],
                        scalar1=2.0,
                        scalar2=0.0,
                        op0=OP.is_lt,
                        op1=OP.add,
                        accum_out=stats[t][:, cb, b:b + 1],
                    )
                if t < T - 1:
                    # W_{t+1} pre-add: (W * 0.5) * m (0 where spiked)
                    nc.vector.scalar_tensor_tensor(
                        out=W[(t + 1) % 2][cb][:], in0=cur[cb][:], scalar=0.5,
                        in1=m[:], op0=OP.mult, op1=OP.mult,
                    )
            # g rows for this t: g = 1 - stats/N
            nc.vector.tensor_scalar(
                out=gm_all[:, :, t, :], in0=stats[t][:],
                scalar1=-1.0 / N, scalar2=1.0,
                op0=OP.mult, op1=OP.add,
            )

        # h1 rows = g @ w1.T for the 8 local (t, b) rows
        h1_ps = psum_h1.tile([ROWS, CR], F32, tag="h1ps", name="h1_ps")
        for cb in range(CB):
            nc.tensor.matmul(
                h1_ps[:],
                gm_all[:, cb].rearrange("c t b -> c (t b)"),
                w1t_sb[:, cb, :],
                start=(cb == 0),
                stop=(cb == CB - 1),
            )
        h1_sb = tpool.tile([ROWS, CR], F32, tag="h1sb", name="h1_sb")
        nc.vector.tensor_copy(h1_sb[:], h1_ps[:])

        if tail_stage < 1:
            continue
        # ---- AllGather local h1 rows -> all 64 batch rows on every core ----
        cc_in = dram.tile([ROWS, CR], F32, tag="ccin", name="cc_in")
        cc_out = dram.tile([TBALL, CR], F32, tag="ccout", name="cc_out")
        nc.sync.dma_start(cc_in[:], h1_sb[:])
        if single:
            for _slot in range(NCORES):
                nc.sync.dma_start(cc_out[ROWS * _slot:ROWS * (_slot + 1), :],
                                  cc_in[:])
        else:
            nc.gpsimd.collective_compute(
                "AllGather", OP.bypass,
                replica_groups=[list(range(NCORES))],
                ins=[cc_in[:].opt()], outs=[cc_out[:].opt()],
            )
        h1_all = tpool.tile([TBALL, CR], F32, tag="h1all", name="h1_all")
        nc.sync.dma_start(h1_all[:], cc_out[:])

        if tail_stage < 2:
            continue
        # ---- BN1 (stats over the 64 batch rows), in [j, tb] layout ----
        h1T_ps = psum_tr.tile([CR, TBALL], F32, tag="h1Tps", name="h1T_ps")
        nc.tensor.transpose(h1T_ps[:], h1_all[:], ident[:TBALL, :TBALL])
        h1T = tpool.tile([CR, TBALL], F32, tag="h1T", name="h1T")
        nc.vector.tensor_copy(h1T[:], h1T_ps[:])
        st6 = tpool.tile([CR, nc.vector.BN_STATS_DIM], F32, tag="st6", name="st6")
        nc.vector.bn_stats(st6[:], h1T[:])
        mv1 = tpool.tile([CR, nc.vector.BN_AGGR_DIM], F32, tag="mv1", name="mv1")
        nc.vector.bn_aggr(mv1[:], st6[:])
        std1 = tpool.tile([CR, 1], F32, tag="std1", name="std1")
        nc.scalar.activation(std1[:], mv1[:, 1:2], AF.Sqrt, bias=eps_sb[:CR])
        d1 = tpool.tile([CR, 1], F32, tag="d1", name="d1")
        nc.vector.reciprocal(d1[:], std1[:])
        nc.vector.tensor_mul(d1[:], d1[:], g1_sb[:])
        sh1 = tpool.tile([CR, 1], F32, tag="sh1", name="sh1")
        nc.vector.tensor_mul(sh1[:], mv1[:, 0:1], d1[:])
        nc.vector.tensor_sub(sh1[:], be1_sb[:], sh1[:])
        # normalized h1.T with columns permuted from gathered order (m, t, b)
        # into output row order (t, m, b)
        h1nT = tpool.tile([CR, TBALL], F32, tag="h1nT", name="h1nT")
        h1nT_wr = h1nT[:].rearrange("j (t m b) -> j m t b", t=T, m=NCORES, b=BL)
        nc.vector.tensor_scalar(
            out=h1nT_wr, in0=h1T[:], scalar1=d1[:], scalar2=sh1[:],
            op0=OP.mult, op1=OP.add,
        )

        if tail_stage < 3:
            continue
        # ---- h2.T = w2 @ h1n.T in [128c, rows] blocks; BN2 per-partition ----
        h2T_ps = psum_h2.tile([128, CG, TBALL], F32, tag="h2Tps", name="h2T_ps")
        for g in range(CG):
            nc.tensor.matmul(h2T_ps[:, g, :],
                             w2t_sb[:, g * 128:(g + 1) * 128], h1nT[:],
                             start=True, stop=True)
        q2 = tpool.tile([128, CG], F32, tag="q2", name="q2")
        h2sq = tpool.tile([128, TBALL], F32, tag="h2sq", name="h2sq")
        for g in range(CG):
            nc.scalar.activation(h2sq[:], h2T_ps[:, g, :], AF.Square,
                                 accum_out=q2[:, g:g + 1])
        # var2 = E[h2^2] - mu2^2 ; d2 = gamma2/sqrt(var2+eps) ; c2 = beta2 - mu2*d2
        var2 = tpool.tile([128, CG], F32, tag="var2", name="var2")
        nc.vector.scalar_tensor_tensor(
            out=var2[:], in0=q2[:], scalar=1.0 / TBALL, in1=mu2sqg[:],
            op0=OP.mult, op1=OP.subtract,
        )
        std2 = tpool.tile([128, CG], F32, tag="std2", name="std2")
        nc.scalar.activation(std2[:], var2[:], AF.Sqrt, bias=eps_sb[:])
        d2 = tpool.tile([128, CG], F32, tag="d2", name="d2")
        nc.vector.reciprocal(d2[:], std2[:])
        nc.vector.tensor_mul(d2[:], d2[:], g2g_sb[:])
        c2 = tpool.tile([128, CG], F32, tag="c2", name="c2")
        nc.vector.tensor_mul(c2[:], mu2g[:], d2[:])
        nc.vector.tensor_sub(c2[:], be2g_sb[:], c2[:])

        if tail_stage < 4:
            continue
        out_sb = opool.tile([128, CG, TBALL], F32, tag="outsb", name="out_sb")
        for g in range(CG):
            nc.vector.tensor_scalar(
                out=out_sb[:, g, :], in0=h2T_ps[:, g, :],
                scalar1=d2[:, g:g + 1], scalar2=c2[:, g:g + 1],
                op0=OP.mult, op1=OP.add,
            )
        nc.sync.dma_start(out[:].rearrange("g c r -> c g r"), out_sb[:])


_CACHE = {}


def _build(repeat=1, single=False, accum_dma=True, tail_stage=99, **_ignored):
    key = ("nc", repeat, single, accum_dma, tail_stage)
    if key in _CACHE:
        return _CACHE[key]
    from contextlib import ExitStack
    nc = bacc.Bacc("TRN2", target_bir_lowering=False, debug=False,
                   num_devices=1 if single else NCORES)
    with tile.TileContext(nc) as tc, ExitStack() as ctx:
        _emit(tc, ctx, repeat=repeat, single=single, accum_dma=accum_dma,
              tail_stage=tail_stage)
    nc.compile()
    _CACHE[key] = nc
    return nc


def make_in_maps(x, w1, gamma1, beta1, w2, gamma2, beta2):
    x = np.ascontiguousarray(np.asarray(x, dtype=np.float32))
    w1t = np.ascontiguousarray(np.asarray(w1, np.float32).T)
    w2t = np.ascontiguousarray(np.asarray(w2, np.float32).T)
    g1 = np.asarray(gamma1, np.float32).reshape(CR, 1)
    be1 = np.asarray(beta1, np.float32).reshape(CR, 1)
    g2g = np.ascontiguousarray(np.asarray(gamma2, np.float32).reshape(CG, 128).T)
    be2g = np.ascontiguousarray(np.asarray(beta2, np.float32).reshape(CG, 128).T)
    return [
        {
            "x": np.ascontiguousarray(x[:, BL * m:BL * (m + 1)]),
            "w1t": w1t, "w2t": w2t,
            "gamma1": g1, "beta1": be1,
            "gamma2g": g2g, "beta2g": be2g,
        }
        for m in range(NCORES)
    ]


def kernel(x, w1, b1, gamma1, beta1, w2, b2, gamma2, beta2):
    # b1/b2 cancel exactly inside the following batch-norms; unused.
    nc = _build()
    in_maps = make_in_maps(x, w1, gamma1, beta1, w2, gamma2, beta2)
    res = run_bass_kernel_spmd(nc, in_maps, core_ids=list(range(NCORES)))
    o = np.asarray(res.results[0]["out"], np.float32)  # [CG, 128, TBALL]
    # rows are (t, m, b) = global (t, batch); channels are g*128 + c
    full = o.reshape(C, TBALL).T  # [64 rows, 512]
    return np.ascontiguousarray(full.reshape(T, B, C, 1))

